# revision 30
# baseline (speedup 1.0000x reference)
"""3-layer GAT on trn2, 8 NeuronCores, edge-parallel with dst-range sharding.

Strategy (per core c, owning dst nodes [c*2500, (c+1)*2500)):
- Edges bucketed by dst into 20 windows of 125 nodes, padded per-window to a
  multiple of 128 (window sizes are the max over cores so the SPMD program is
  identical on every core). Within a window edges are sorted by src so the
  per-edge table gather reads ascending addresses.
- Dense phase: per window, ONE matmul chain against host-packed [W | W@a_s |
  W@a_d] yields h, alpha_src, alpha_dst together in PSUM. Rows are packed
  fp16 as [h0|1|h1|1|h2|1|h3|1|as(4)|ad(4)|pad] (384 cols; layer 3:
  [h|1|as|ad|pad], 128 cols) and AllGathered so every core holds the full
  20000-row table.
- Edge phase: per window, gather-1 pulls full 768B rows for edge SOURCES,
  gather-2 pulls the 256B tail of rows for edge DESTINATIONS (for alpha_dst).
  Scores e = leaky(as+ad), p = exp(e-3) are computed window-batched. Per
  128-edge chunk a dst one-hot (iota==dstloc) and one broadcast multiply
  rhs = p (.) [h|1] feed a scatter matmul accumulating [sum p*h | sum p] per
  head into PSUM. Window epilogue normalizes; bias+relu ride the PSUM->SBUF
  copy of the transposed activations for the next layer.
"""
import os, sys
for _p in ('/opt/trn_rl_repo', '/root/.axon_site/_ro/trn_rl_repo'):
    if os.path.isdir(_p) and _p not in sys.path:
        sys.path.insert(0, _p)

import numpy as np

import concourse.bacc as bacc
import concourse.tile as tile
from concourse import bass, mybir
from concourse import bass_utils

N = 20000
E = 320000
HID = 64
HEADS = 4
OUT_CH = 64
NEG = 0.2
C = 8
SHARD = N // C          # 2500
WIN = 125               # dst nodes per window
NW = SHARD // WIN       # 20
P = 128
EXP_BIAS = -3.0         # constant shift inside exp; cancels in the softmax

# fin, fout, heads, fp16 table row width (128-multiple >= fout+2*heads+heads)
LAYERS = [
    dict(fin=64,  fout=256, heads=4, row=384),
    dict(fin=256, fout=256, heads=4, row=384),
    dict(fin=256, fout=64,  heads=1, row=128),
]

AX = mybir.AxisListType
ALU = mybir.AluOpType
ACTF = mybir.ActivationFunctionType
F32 = mybir.dt.float32
F16 = mybir.dt.float16
I16 = mybir.dt.int16


def _host_prep(edge_index):
    """Per-core src/dst gather-index + dstloc arrays and shared window sizes."""
    src = np.asarray(edge_index[0], dtype=np.int64)
    dst = np.asarray(edge_index[1], dtype=np.int64)
    per_core = []   # (srcs, dsts, dstloc) per (core, window)
    counts = np.zeros((C, NW), dtype=np.int64)
    for c in range(C):
        m = (dst >= c * SHARD) & (dst < (c + 1) * SHARD)
        es, ed = src[m], dst[m] - c * SHARD
        w = ed // WIN
        wins = []
        for wi in range(NW):
            sel = w == wi
            ws, wd = es[sel], ed[sel] - wi * WIN
            order = np.argsort(ws, kind='stable')   # src-sorted for DMA locality
            wins.append((ws[order], wd[order]))
            counts[c, wi] = sel.sum()
        per_core.append(wins)
    kws = (np.ceil(counts.max(axis=0) / P).astype(np.int64) * P)
    kws = np.maximum(kws, P)
    tot = int(kws.sum())
    isrc_all, idst_all, dl_all = [], [], []
    for c in range(C):
        isrc_mat = np.zeros((16, tot // 16), dtype=np.int16)
        idst_mat = np.zeros((16, tot // 16), dtype=np.int16)
        dl_mat = np.full((P, tot // P), float(WIN), dtype=np.float32)
        icol = ccol = 0
        for wi in range(NW):
            kw = int(kws[wi])
            es, dl = per_core[c][wi]
            n = len(es)
            sp = np.zeros(kw, dtype=np.int16)
            tp = np.zeros(kw, dtype=np.int16)
            dp = np.full(kw, float(WIN), dtype=np.float32)
            sp[:n] = es.astype(np.int16)
            # dst node GLOBAL id for gather-2 (alpha_dst rows live in tab_f)
            tp[:n] = (dl + wi * WIN + c * SHARD).astype(np.int16)
            dp[:n] = dl.astype(np.float32)
            isrc_mat[:, icol:icol + kw // 16] = sp.reshape(-1, 16).T
            idst_mat[:, icol:icol + kw // 16] = tp.reshape(-1, 16).T
            dl_mat[:, ccol:ccol + kw // P] = dp.reshape(-1, P).T
            icol += kw // 16
            ccol += kw // P
        isrc_all.append(np.tile(isrc_mat, (8, 1)))
        idst_all.append(np.tile(idst_mat, (8, 1)))
        dl_all.append(dl_mat)
    return tuple(int(k) for k in kws), isrc_all, idst_all, dl_all


def build(kws, timing_reps=0):
    """Builds the SPMD bass module. kws: per-window padded edge counts."""
    tot = sum(kws)
    nc = bacc.Bacc("TRN2", target_bir_lowering=False, debug=False, num_devices=C)

    # ---- DRAM I/O ----
    d_xT = nc.dram_tensor("xT_own", [HID, SHARD], F16, kind="ExternalInput")
    d_W = [nc.dram_tensor(f"Wp{l+1}",
                          [LAYERS[l]['fin'], LAYERS[l]['fout'] + 2 * LAYERS[l]['heads']],
                          F16, kind="ExternalInput") for l in range(3)]
    d_btT = [nc.dram_tensor(f"btT{l+1}", [P, LAYERS[l]['fout'] // P], F32,
                            kind="ExternalInput") for l in range(2)]
    d_bt3 = nc.dram_tensor("bt3", [P, OUT_CH], F32, kind="ExternalInput")
    d_iota = nc.dram_tensor("iota32", [P, P], F32, kind="ExternalInput")
    d_ident = nc.dram_tensor("ident", [P, P], F32, kind="ExternalInput")
    d_isrc = nc.dram_tensor("idx_src", [P, tot // 16], I16, kind="ExternalInput")
    d_idst = nc.dram_tensor("idx_dst", [P, tot // 16], I16, kind="ExternalInput")
    d_dl = nc.dram_tensor("dl32", [P, tot // P], F32, kind="ExternalInput")
    d_out = nc.dram_tensor("out", [SHARD, OUT_CH], F32, kind="ExternalOutput")
    if timing_reps:
        d_tok = nc.dram_tensor("tok", [1, 32], F32, kind="ExternalInput")
        d_toko = nc.dram_tensor("tok_out", [1, 32], F32, kind="ExternalOutput")

    tabs = []
    for l, cfg in enumerate(LAYERS):
        s = nc.dram_tensor(f"tab{l+1}s", [SHARD, cfg['row']], F16)
        f = nc.dram_tensor(f"tab{l+1}f", [N, cfg['row']], F16, addr_space="Shared")
        tabs.append((s, f))

    with tile.TileContext(nc) as tc:
        with tc.tile_pool(name="const", bufs=1) as cp, \
             tc.tile_pool(name="gp", bufs=3) as gp, \
             tc.tile_pool(name="sp", bufs=3) as sp, \
             tc.tile_pool(name="ohp", bufs=20) as ohp, \
             tc.tile_pool(name="ohtp", bufs=20) as ohtp, \
             tc.tile_pool(name="rp", bufs=3) as rp, \
             tc.tile_pool(name="op", bufs=2) as op_, \
             tc.tile_pool(name="ps", bufs=1, space="PSUM") as pp:

            # ---- persistent SBUF ----
            iota = cp.tile([P, P], F32)
            ident = cp.tile([P, P], F32)
            nc.sync.dma_start(iota[:], d_iota[:, :])
            nc.sync.dma_start(ident[:], d_ident[:, :])
            isrc_sb = cp.tile([P, tot // 16], I16)
            dl_sb = cp.tile([P, tot // P], F32)
            nc.sync.dma_start(isrc_sb[:], d_isrc[:, :])
            nc.sync.dma_start(dl_sb[:], d_dl[:, :])
            xT = cp.tile([HID, SHARD], F16)
            nc.sync.dma_start(xT[:], d_xT[:, :])
            Wt, btTt = [], []
            for l, cfg in enumerate(LAYERS):
                fin, wcols = cfg['fin'], cfg['fout'] + 2 * cfg['heads']
                chunks = []
                for kc in range(0, fin, P):
                    ke = min(kc + P, fin)
                    t = cp.tile([ke - kc, wcols], F16, tag=f"W{l}_{kc}")
                    nc.sync.dma_start(t[:], d_W[l][kc:ke, :])
                    chunks.append(t)
                Wt.append(chunks)
            for l in range(2):
                t = cp.tile([P, LAYERS[l]['fout'] // P], F32, tag=f"btT{l}")
                nc.sync.dma_start(t[:], d_btT[l][:, :])
                btTt.append(t)
            bt3 = cp.tile([P, OUT_CH], F32)
            nc.sync.dma_start(bt3[:], d_bt3[:, :])
            ebias = cp.tile([P, 1], F32)
            nc.vector.memset(ebias[:], EXP_BIAS)
            ident16 = cp.tile([P, P], F16)
            nc.vector.tensor_copy(ident16[:], ident[:])
            actT = {1: [cp.tile([P, SHARD], F16, tag=f"actT1_{j}", name=f"actT1_{j}")
                        for j in range(2)],
                    2: [cp.tile([P, SHARD], F16, tag=f"actT2_{j}", name=f"actT2_{j}")
                        for j in range(2)]}
            # per-layer local alpha_dst for own dst nodes (window-major)
            ado16 = [cp.tile([WIN, NW * LAYERS[l]['heads']], F16, tag=f"ado{l}",
                             name=f"ado{l}") for l in range(3)]
            # two row-staging tiles per layer; ones/pad columns set once here
            rows = []
            for l, cfg in enumerate(LAYERS):
                heads, row = cfg['heads'], cfg['row']
                pair = []
                for j in range(2):
                    rt = cp.tile([P, row], F16, tag=f"row{l}_{j}", name=f"row{l}_{j}")
                    nc.vector.memset(rt[:], 0.0)
                    nc.vector.memset(
                        rt[:, 0:65 * heads].rearrange("p (h c) -> p h c", c=65)[:, :, 64:65],
                        1.0)
                    pair.append(rt)
                rows.append(pair)

            def dense_phase(l, actT_in):
                cfg = LAYERS[l]
                fout, heads = cfg['fout'], cfg['heads']
                tab_s = tabs[l][0]
                nchunks = len(Wt[l])
                for w in range(NW):
                    ph = pp.tile([WIN, fout + 2 * heads], F32, tag="ph", bufs=2)
                    for kc in range(nchunks):
                        nc.tensor.matmul(
                            ph[:, :], lhsT=actT_in[kc][:, w * WIN:(w + 1) * WIN],
                            rhs=Wt[l][kc][:], start=(kc == 0), stop=(kc == nchunks - 1))
                    row_t = rows[l][w % 2]
                    # h columns (strided past the interleaved ones columns)
                    nc.vector.tensor_copy(
                        row_t[:WIN, 0:65 * heads].rearrange(
                            "p (h c) -> p h c", c=65)[:, :, 0:64],
                        ph[:, 0:fout].rearrange("p (h c) -> p h c", c=64))
                    # as (gathered per edge); ad stays local in SBUF
                    nc.vector.tensor_copy(
                        row_t[:WIN, 65 * heads:65 * heads + heads],
                        ph[:, fout:fout + heads])
                    nc.vector.tensor_copy(
                        ado16[l][:, w * heads:(w + 1) * heads],
                        ph[:, fout + heads:fout + 2 * heads])
                    nc.sync.dma_start(tab_s[w * WIN:(w + 1) * WIN, :], row_t[:WIN, :])

            def allgather(l):
                tab_s, tab_f = tabs[l]
                if timing_reps:
                    nshard = C if os.environ.get("GAT_AG_MODE", "mock8") == "mock8" else 1
                    for s in range(nshard):
                        nc.sync.dma_start(tab_f[s * SHARD:(s + 1) * SHARD, :],
                                          tab_s[:, :])
                else:
                    nc.gpsimd.collective_compute(
                        "AllGather", ALU.bypass,
                        replica_groups=[list(range(C))],
                        ins=[tab_s[:, :]], outs=[tab_f[:, :]])

            def edge_phase(l, actT_next):
                ep = int(os.environ.get("GAT_EP", "9"))
                spkt = os.environ.get("GAT_SP", "0") == "1"
                cfg = LAYERS[l]
                fout, heads, row = cfg['fout'], cfg['heads'], cfg['row']
                ncols = 65 * heads          # 260 or 65
                g2off = row - P             # 256 or 0
                adrel = ncols - g2off + heads
                tab_f = tabs[l][1]
                icol = ccol = 0
                for w in range(NW):
                    kw = kws[w]
                    tw = kw // P
                    twb = max(tw // 3, 1) if tw > 1 else 0
                    twa = tw - twb
                    # two half-window gathers into SEPARATE tiles: scores and
                    # scatter for half A run while half B is still in flight
                    g1a = gp.tile([P, twa * row], F16, tag="g1a")
                    nc.gpsimd.dma_gather(
                        g1a[:].rearrange("p (t e) -> p t e", e=row), tab_f[:, :],
                        isrc_sb[:, icol:icol + twa * 8],
                        twa * P, twa * P, row, single_packet=spkt)
                    if twb:
                        g1b = gp.tile([P, twb * row], F16, tag="g1b")
                        nc.gpsimd.dma_gather(
                            g1b[:].rearrange("p (t e) -> p t e", e=row), tab_f[:, :],
                            isrc_sb[:, icol + twa * 8:icol + tw * 8],
                            twb * P, twb * P, row, single_packet=spkt)
                    if ep < 3:
                        icol += kw // 16
                        ccol += tw
                        continue
                    ad_win = ado16[l][:, w * heads:(w + 1) * heads]
                    # per-edge alpha_dst: transpose the dst one-hot on PE, copy
                    # PSUM->SBUF on the scalar engine, then a small matmul
                    # against the window's local alpha_dst rows
                    pads = pp.tile([P, tw * heads], F32, tag="pads", bufs=1)
                    ohs, ohT16s = [], []
                    for t in range(tw):
                        dcol = dl_sb[:, ccol + t:ccol + t + 1]
                        oh = ohp.tile([P, WIN], F16, tag="oh")
                        nc.vector.tensor_scalar(oh[:], iota[:, :WIN], dcol, None,
                                                op0=ALU.is_equal)
                        ohs.append(oh)
                        ohT_ps = pp.tile([WIN, P], F16, tag="ohT", bufs=2)
                        nc.tensor.transpose(ohT_ps[:], oh[:], ident16[:, :])
                        ohT16 = ohtp.tile([WIN, P], F16, tag="ohT16")
                        nc.scalar.activation(ohT16[:], ohT_ps[:], ACTF.Copy)
                        ohT16s.append(ohT16)
                    for t in range(tw):
                        nc.tensor.matmul(pads[:, t * heads:(t + 1) * heads],
                                         lhsT=ohT16s[t][:], rhs=ad_win,
                                         start=True, stop=True)
                    if ep < 4:
                        icol += kw // 16
                        ccol += tw
                        continue
                    psw = pp.tile([WIN, ncols], F32, tag="psw", bufs=2)
                    halves = [(g1a, 0, twa)]
                    if twb:
                        halves.append((g1b, twa, twb))
                    for gt, t0, twh in halves:
                        # per-half scores: p = exp(leaky(as+ad) - 3)
                        gt_3 = gt[:].rearrange("p (t e) -> p t e", e=row)
                        st = sp.tile([P, twh * heads], F32, tag="st")
                        st3 = st[:].rearrange("p (t h) -> p t h", h=heads)
                        nc.vector.tensor_tensor(
                            st3, gt_3[:, :, ncols:ncols + heads],
                            pads[:, t0 * heads:(t0 + twh) * heads].rearrange(
                                "p (t h) -> p t h", h=heads), op=ALU.add)
                        lt = sp.tile([P, twh * heads], F32, tag="lt")
                        nc.vector.tensor_scalar(lt[:], st[:], NEG, None, op0=ALU.mult)
                        nc.vector.tensor_tensor(lt[:], lt[:], st[:], op=ALU.max)
                        p16 = sp.tile([P, twh * heads], F16, tag="p16")
                        nc.scalar.activation(p16[:], lt[:], ACTF.Exp,
                                             bias=ebias[:, 0:1])
                        for tt in range(twh):
                            t = t0 + tt
                            rhs_t = rp.tile([P, ncols], F16, tag="rhs")
                            nc.vector.tensor_tensor(
                                rhs_t[:].rearrange("p (h c) -> p h c", c=65),
                                gt[:, tt * row:tt * row + ncols].rearrange(
                                    "p (h c) -> p h c", c=65),
                                p16[:, tt * heads:(tt + 1) * heads].broadcast_to(
                                    (P, heads, 65)),
                                op=ALU.mult)
                            nc.tensor.matmul(psw[:], lhsT=ohs[t][:], rhs=rhs_t[:],
                                             start=(t == 0), stop=(t == tw - 1))
                    if ep < 5:
                        icol += kw // 16
                        ccol += tw
                        continue
                    # window epilogue: normalize, bias(+relu), transpose for next
                    psw_v = psw[:].rearrange("p (h c) -> p h c", c=65)
                    den = sp.tile([WIN, heads], F32, tag="den")
                    nc.vector.tensor_scalar(
                        den[:].rearrange("p (h c) -> p h c", c=1),
                        psw_v[:, :, 64:65], 1e-16, None, op0=ALU.add)
                    rec = sp.tile([WIN, heads], F32, tag="rec")
                    nc.vector.reciprocal(rec[:], den[:])
                    if l < 2:
                        orow = op_.tile([P, fout], F32, tag="orow")
                        nc.vector.tensor_tensor(
                            orow[:WIN, :].rearrange("p (h c) -> p h c", c=64),
                            psw_v[:, :, 0:64],
                            rec[:, :].broadcast_to((WIN, heads, 64)), op=ALU.mult)
                        for j in range(fout // P):
                            pt = pp.tile([P, WIN], F32, tag="pt", bufs=1)
                            nc.tensor.transpose(pt[:], orow[:WIN, j * P:(j + 1) * P],
                                                ident[:WIN, :WIN])
                            # bias + relu ride the PSUM->SBUF copy
                            nc.vector.tensor_scalar(
                                actT_next[j][:, w * WIN:(w + 1) * WIN],
                                pt[:, :WIN], btTt[l][:, j:j + 1], 0.0,
                                op0=ALU.add, op1=ALU.max)
                    else:
                        orow = op_.tile([P, OUT_CH], F32, tag="orow")
                        nc.vector.tensor_scalar(orow[:WIN, :], psw[:, 0:OUT_CH],
                                                rec[:, 0:1], None, op0=ALU.mult)
                        nc.vector.tensor_tensor(orow[:WIN, :], orow[:WIN, :],
                                                bt3[:WIN, :], op=ALU.add)
                        nc.sync.dma_start(d_out[w * WIN:(w + 1) * WIN, :],
                                          orow[:WIN, :])
                    icol += kw // 16
                    ccol += tw

            def body():
                stages = int(os.environ.get("GAT_STAGES", "9"))  # 9 = full network
                dense_phase(0, [xT])
                if stages >= 2:
                    allgather(0)
                if stages >= 3:
                    edge_phase(0, actT[1])
                if stages >= 4:
                    dense_phase(1, actT[1])
                    allgather(1)
                if stages >= 5:
                    edge_phase(1, actT[2])
                if stages >= 6:
                    dense_phase(2, actT[2])
                    allgather(2)
                if stages >= 7:
                    edge_phase(2, None)
                if stages < 7:
                    z = op_.tile([WIN, OUT_CH], F32, tag="z", name="z")
                    nc.vector.memset(z[:], 0.0)
                    for w in range(NW):
                        nc.sync.dma_start(d_out[w * WIN:(w + 1) * WIN, :], z[:])

            if timing_reps:
                tk = cp.tile([1, 32], F32)
                nc.sync.dma_start(tk[:], d_tok[:, :])
                if timing_reps == 1:
                    body()
                else:
                    with tc.For_i(0, timing_reps, 1):
                        body()
                nc.sync.dma_start(d_toko[:, :], tk[:])
            else:
                body()

    nc.compile()
    return nc


def _host_inputs(x, edge_index, W1, a1s, a1d, b1, W2, a2s, a2d, b2, W3, a3s, a3d, b3):
    kws, isrc_all, idst_all, dl_all = _host_prep(edge_index)
    x = np.asarray(x, dtype=np.float32)
    Ws = [np.asarray(W1, np.float32), np.asarray(W2, np.float32),
          np.asarray(W3, np.float32)]
    As = [np.asarray(a1s, np.float32), np.asarray(a2s, np.float32),
          np.asarray(a3s, np.float32)]
    Ad = [np.asarray(a1d, np.float32), np.asarray(a2d, np.float32),
          np.asarray(a3d, np.float32)]
    bs = [np.asarray(b1, np.float32), np.asarray(b2, np.float32),
          np.asarray(b3, np.float32)]
    shared = {}
    for l in range(3):
        fout, heads = LAYERS[l]['fout'], LAYERS[l]['heads']
        dh = fout // heads
        # Wa[:, h] = W[:, h*dh:(h+1)*dh] @ a[h]  (as/ad folded into the dense matmul)
        W3d = Ws[l].reshape(-1, heads, dh)
        Was = np.einsum('ihd,hd->ih', W3d, As[l])
        Wad = np.einsum('ihd,hd->ih', W3d, Ad[l])
        shared[f"Wp{l+1}"] = np.concatenate([Ws[l], Was, Wad], axis=1).astype(np.float16)
    for l in range(2):
        shared[f"btT{l+1}"] = np.ascontiguousarray(
            bs[l].reshape(-1, P).T).astype(np.float32)
    shared["bt3"] = np.tile(bs[2].reshape(1, OUT_CH), (P, 1)).astype(np.float32)
    shared["iota32"] = np.tile(np.arange(P, dtype=np.float32).reshape(1, P), (P, 1))
    shared["ident"] = np.eye(P, dtype=np.float32)
    in_maps = []
    for c in range(C):
        m = dict(shared)
        m["xT_own"] = np.ascontiguousarray(
            x[c * SHARD:(c + 1) * SHARD].T).astype(np.float16)
        m["idx_src"] = isrc_all[c]
        m["idx_dst"] = idst_all[c]
        m["dl32"] = dl_all[c]
        in_maps.append(m)
    return kws, in_maps


_CACHE = {}


def kernel(**inputs) -> np.ndarray:
    kws, in_maps = _host_inputs(**inputs)
    if kws not in _CACHE:
        _CACHE[kws] = build(kws)
    nc = _CACHE[kws]
    last = None
    for _attempt in range(2):
        try:
            res = bass_utils.run_bass_kernel_spmd(
                nc, in_maps, core_ids=list(range(C)), trace=False)
            return np.concatenate(
                [res.results[c]["out"] for c in range(C)], axis=0)
        except Exception as e:  # rare transient device-mesh hiccups: retry once
            last = e
    raise last


# revision 32
# speedup vs baseline: 1.0224x; 1.0224x over previous
"""3-layer GAT on trn2, 8 NeuronCores, edge-parallel with dst-range sharding.

Strategy (per core c, owning dst nodes [c*2500, (c+1)*2500)):
- Edges bucketed by dst into 20 windows of 125 nodes, padded per-window to a
  multiple of 128 (window sizes are the max over cores so the SPMD program is
  identical on every core). Within a window edges are sorted by src so the
  per-edge table gather reads ascending addresses.
- Dense phase: per window, ONE matmul chain against host-packed [W | W@a_s |
  W@a_d] yields h, alpha_src, alpha_dst together in PSUM. Rows are packed
  fp16 as [h0|1|h1|1|h2|1|h3|1|as(4)|ad(4)|pad] (384 cols; layer 3:
  [h|1|as|ad|pad], 128 cols) and AllGathered so every core holds the full
  20000-row table.
- Edge phase: per window, gather-1 pulls full 768B rows for edge SOURCES,
  gather-2 pulls the 256B tail of rows for edge DESTINATIONS (for alpha_dst).
  Scores e = leaky(as+ad), p = exp(e-3) are computed window-batched. Per
  128-edge chunk a dst one-hot (iota==dstloc) and one broadcast multiply
  rhs = p (.) [h|1] feed a scatter matmul accumulating [sum p*h | sum p] per
  head into PSUM. Window epilogue normalizes; bias+relu ride the PSUM->SBUF
  copy of the transposed activations for the next layer.
"""
import os, sys
for _p in ('/opt/trn_rl_repo', '/root/.axon_site/_ro/trn_rl_repo'):
    if os.path.isdir(_p) and _p not in sys.path:
        sys.path.insert(0, _p)

import numpy as np

import concourse.bacc as bacc
import concourse.tile as tile
from concourse import bass, mybir
from concourse import bass_utils

N = 20000
E = 320000
HID = 64
HEADS = 4
OUT_CH = 64
NEG = 0.2
C = 8
SHARD = N // C          # 2500
WIN = 125               # dst nodes per window
NW = SHARD // WIN       # 20
P = 128
EXP_BIAS = -3.0         # constant shift inside exp; cancels in the softmax

# fin, fout, heads, fp16 table row width (128-multiple >= fout+2*heads+heads)
LAYERS = [
    dict(fin=64,  fout=256, heads=4, row=384),
    dict(fin=256, fout=256, heads=4, row=384),
    dict(fin=256, fout=64,  heads=1, row=128),
]

AX = mybir.AxisListType
ALU = mybir.AluOpType
ACTF = mybir.ActivationFunctionType
F32 = mybir.dt.float32
F16 = mybir.dt.float16
I16 = mybir.dt.int16


def _host_prep(edge_index):
    """Per-core src/dst gather-index + dstloc arrays and shared window sizes."""
    src = np.asarray(edge_index[0], dtype=np.int64)
    dst = np.asarray(edge_index[1], dtype=np.int64)
    per_core = []   # (srcs, dsts, dstloc) per (core, window)
    counts = np.zeros((C, NW), dtype=np.int64)
    for c in range(C):
        m = (dst >= c * SHARD) & (dst < (c + 1) * SHARD)
        es, ed = src[m], dst[m] - c * SHARD
        w = ed // WIN
        wins = []
        for wi in range(NW):
            sel = w == wi
            ws, wd = es[sel], ed[sel] - wi * WIN
            order = np.argsort(ws, kind='stable')   # src-sorted for DMA locality
            wins.append((ws[order], wd[order]))
            counts[c, wi] = sel.sum()
        per_core.append(wins)
    kws = (np.ceil(counts.max(axis=0) / P).astype(np.int64) * P)
    kws = np.maximum(kws, P)
    tot = int(kws.sum())
    isrc_all, idst_all, dl_all = [], [], []
    for c in range(C):
        isrc_mat = np.zeros((16, tot // 16), dtype=np.int16)
        idst_mat = np.zeros((16, tot // 16), dtype=np.int16)
        dl_mat = np.full((P, tot // P), float(WIN), dtype=np.float32)
        icol = ccol = 0
        for wi in range(NW):
            kw = int(kws[wi])
            es, dl = per_core[c][wi]
            n = len(es)
            sp = np.zeros(kw, dtype=np.int16)
            tp = np.zeros(kw, dtype=np.int16)
            dp = np.full(kw, float(WIN), dtype=np.float32)
            sp[:n] = es.astype(np.int16)
            # dst node GLOBAL id for gather-2 (alpha_dst rows live in tab_f)
            tp[:n] = (dl + wi * WIN + c * SHARD).astype(np.int16)
            dp[:n] = dl.astype(np.float32)
            isrc_mat[:, icol:icol + kw // 16] = sp.reshape(-1, 16).T
            idst_mat[:, icol:icol + kw // 16] = tp.reshape(-1, 16).T
            dl_mat[:, ccol:ccol + kw // P] = dp.reshape(-1, P).T
            icol += kw // 16
            ccol += kw // P
        isrc_all.append(np.tile(isrc_mat, (8, 1)))
        idst_all.append(np.tile(idst_mat, (8, 1)))
        dl_all.append(dl_mat)
    return tuple(int(k) for k in kws), isrc_all, idst_all, dl_all


def build(kws, timing_reps=0):
    """Builds the SPMD bass module. kws: per-window padded edge counts."""
    tot = sum(kws)
    nc = bacc.Bacc("TRN2", target_bir_lowering=False, debug=False, num_devices=C)

    # ---- DRAM I/O ----
    d_xT = nc.dram_tensor("xT_own", [HID, SHARD], F16, kind="ExternalInput")
    d_W = [nc.dram_tensor(f"Wp{l+1}",
                          [LAYERS[l]['fin'], LAYERS[l]['fout'] + 2 * LAYERS[l]['heads']],
                          F16, kind="ExternalInput") for l in range(3)]
    d_btT = [nc.dram_tensor(f"btT{l+1}", [P, LAYERS[l]['fout'] // P], F32,
                            kind="ExternalInput") for l in range(2)]
    d_bt3 = nc.dram_tensor("bt3", [P, OUT_CH], F32, kind="ExternalInput")
    d_iota = nc.dram_tensor("iota32", [P, P], F32, kind="ExternalInput")
    d_ident = nc.dram_tensor("ident", [P, P], F32, kind="ExternalInput")
    d_isrc = nc.dram_tensor("idx_src", [P, tot // 16], I16, kind="ExternalInput")
    d_idst = nc.dram_tensor("idx_dst", [P, tot // 16], I16, kind="ExternalInput")
    d_dl = nc.dram_tensor("dl32", [P, tot // P], F32, kind="ExternalInput")
    d_out = nc.dram_tensor("out", [SHARD, OUT_CH], F32, kind="ExternalOutput")
    if timing_reps:
        d_tok = nc.dram_tensor("tok", [1, 32], F32, kind="ExternalInput")
        d_toko = nc.dram_tensor("tok_out", [1, 32], F32, kind="ExternalOutput")

    tabs = []
    for l, cfg in enumerate(LAYERS):
        s = nc.dram_tensor(f"tab{l+1}s", [SHARD, cfg['row']], F16)
        f = nc.dram_tensor(f"tab{l+1}f", [N, cfg['row']], F16, addr_space="Shared")
        tabs.append((s, f))

    with tile.TileContext(nc) as tc:
        with tc.tile_pool(name="const", bufs=1) as cp, \
             tc.tile_pool(name="gp", bufs=4) as gp, \
             tc.tile_pool(name="sp", bufs=4) as sp, \
             tc.tile_pool(name="ohp", bufs=20) as ohp, \
             tc.tile_pool(name="ohtp", bufs=20) as ohtp, \
             tc.tile_pool(name="rp", bufs=6) as rp, \
             tc.tile_pool(name="op", bufs=3) as op_, \
             tc.tile_pool(name="ps", bufs=1, space="PSUM") as pp:

            # ---- persistent SBUF ----
            iota = cp.tile([P, P], F32)
            ident = cp.tile([P, P], F32)
            nc.sync.dma_start(iota[:], d_iota[:, :])
            nc.sync.dma_start(ident[:], d_ident[:, :])
            isrc_sb = cp.tile([P, tot // 16], I16)
            dl_sb = cp.tile([P, tot // P], F32)
            nc.sync.dma_start(isrc_sb[:], d_isrc[:, :])
            nc.sync.dma_start(dl_sb[:], d_dl[:, :])
            xT = cp.tile([HID, SHARD], F16)
            nc.sync.dma_start(xT[:], d_xT[:, :])
            Wt, btTt = [], []
            for l, cfg in enumerate(LAYERS):
                fin, wcols = cfg['fin'], cfg['fout'] + 2 * cfg['heads']
                chunks = []
                for kc in range(0, fin, P):
                    ke = min(kc + P, fin)
                    t = cp.tile([ke - kc, wcols], F16, tag=f"W{l}_{kc}")
                    nc.sync.dma_start(t[:], d_W[l][kc:ke, :])
                    chunks.append(t)
                Wt.append(chunks)
            for l in range(2):
                t = cp.tile([P, LAYERS[l]['fout'] // P], F32, tag=f"btT{l}")
                nc.sync.dma_start(t[:], d_btT[l][:, :])
                btTt.append(t)
            bt3 = cp.tile([P, OUT_CH], F32)
            nc.sync.dma_start(bt3[:], d_bt3[:, :])
            ebias = cp.tile([P, 1], F32)
            nc.vector.memset(ebias[:], EXP_BIAS)
            ident16 = cp.tile([P, P], F16)
            nc.vector.tensor_copy(ident16[:], ident[:])
            actT = {1: [cp.tile([P, SHARD], F16, tag=f"actT1_{j}", name=f"actT1_{j}")
                        for j in range(2)],
                    2: [cp.tile([P, SHARD], F16, tag=f"actT2_{j}", name=f"actT2_{j}")
                        for j in range(2)]}
            # per-layer local alpha_dst for own dst nodes (window-major)
            ado16 = [cp.tile([WIN, NW * LAYERS[l]['heads']], F16, tag=f"ado{l}",
                             name=f"ado{l}") for l in range(3)]
            # two row-staging tiles per layer; ones/pad columns set once here
            rows = []
            for l, cfg in enumerate(LAYERS):
                heads, row = cfg['heads'], cfg['row']
                pair = []
                for j in range(2):
                    rt = cp.tile([P, row], F16, tag=f"row{l}_{j}", name=f"row{l}_{j}")
                    nc.vector.memset(rt[:], 0.0)
                    nc.vector.memset(
                        rt[:, 0:65 * heads].rearrange("p (h c) -> p h c", c=65)[:, :, 64:65],
                        1.0)
                    pair.append(rt)
                rows.append(pair)

            def dense_phase(l, actT_in):
                cfg = LAYERS[l]
                fout, heads = cfg['fout'], cfg['heads']
                tab_s = tabs[l][0]
                nchunks = len(Wt[l])
                for w in range(NW):
                    ph = pp.tile([WIN, fout + 2 * heads], F32, tag="ph", bufs=2)
                    for kc in range(nchunks):
                        nc.tensor.matmul(
                            ph[:, :], lhsT=actT_in[kc][:, w * WIN:(w + 1) * WIN],
                            rhs=Wt[l][kc][:], start=(kc == 0), stop=(kc == nchunks - 1))
                    row_t = rows[l][w % 2]
                    # h columns (strided past the interleaved ones columns)
                    nc.vector.tensor_copy(
                        row_t[:WIN, 0:65 * heads].rearrange(
                            "p (h c) -> p h c", c=65)[:, :, 0:64],
                        ph[:, 0:fout].rearrange("p (h c) -> p h c", c=64))
                    # as (gathered per edge); ad stays local in SBUF
                    nc.vector.tensor_copy(
                        row_t[:WIN, 65 * heads:65 * heads + heads],
                        ph[:, fout:fout + heads])
                    nc.vector.tensor_copy(
                        ado16[l][:, w * heads:(w + 1) * heads],
                        ph[:, fout + heads:fout + 2 * heads])
                    nc.sync.dma_start(tab_s[w * WIN:(w + 1) * WIN, :], row_t[:WIN, :])

            def allgather(l):
                tab_s, tab_f = tabs[l]
                if timing_reps:
                    nshard = C if os.environ.get("GAT_AG_MODE", "mock8") == "mock8" else 1
                    for s in range(nshard):
                        nc.sync.dma_start(tab_f[s * SHARD:(s + 1) * SHARD, :],
                                          tab_s[:, :])
                else:
                    nc.gpsimd.collective_compute(
                        "AllGather", ALU.bypass,
                        replica_groups=[list(range(C))],
                        ins=[tab_s[:, :]], outs=[tab_f[:, :]])

            def edge_phase(l, actT_next):
                ep = int(os.environ.get("GAT_EP", "9"))
                spkt = os.environ.get("GAT_SP", "0") == "1"
                cfg = LAYERS[l]
                fout, heads, row = cfg['fout'], cfg['heads'], cfg['row']
                ncols = 65 * heads          # 260 or 65
                g2off = row - P             # 256 or 0
                adrel = ncols - g2off + heads
                tab_f = tabs[l][1]
                icol = ccol = 0
                for w in range(NW):
                    kw = kws[w]
                    tw = kw // P
                    twa = (tw + 1) // 2
                    twb = tw - twa
                    # two half-window gathers into SEPARATE tiles: scores and
                    # scatter for half A run while half B is still in flight
                    g1a = gp.tile([P, twa * row], F16, tag="g1a")
                    nc.gpsimd.dma_gather(
                        g1a[:].rearrange("p (t e) -> p t e", e=row), tab_f[:, :],
                        isrc_sb[:, icol:icol + twa * 8],
                        twa * P, twa * P, row, single_packet=spkt)
                    if twb:
                        g1b = gp.tile([P, twb * row], F16, tag="g1b")
                        nc.gpsimd.dma_gather(
                            g1b[:].rearrange("p (t e) -> p t e", e=row), tab_f[:, :],
                            isrc_sb[:, icol + twa * 8:icol + tw * 8],
                            twb * P, twb * P, row, single_packet=spkt)
                    if ep < 3:
                        icol += kw // 16
                        ccol += tw
                        continue
                    ad_win = ado16[l][:, w * heads:(w + 1) * heads]
                    # per-edge alpha_dst: transpose the dst one-hot on PE, copy
                    # PSUM->SBUF on the scalar engine, then a small matmul
                    # against the window's local alpha_dst rows
                    pads = pp.tile([P, tw * heads], F32, tag="pads", bufs=1)
                    ohs, ohT16s = [], []
                    for t in range(tw):
                        dcol = dl_sb[:, ccol + t:ccol + t + 1]
                        oh = ohp.tile([P, WIN], F16, tag="oh")
                        nc.vector.tensor_scalar(oh[:], iota[:, :WIN], dcol, None,
                                                op0=ALU.is_equal)
                        ohs.append(oh)
                        ohT_ps = pp.tile([WIN, P], F16, tag="ohT", bufs=2)
                        nc.tensor.transpose(ohT_ps[:], oh[:], ident16[:, :])
                        ohT16 = ohtp.tile([WIN, P], F16, tag="ohT16")
                        nc.scalar.activation(ohT16[:], ohT_ps[:], ACTF.Copy)
                        ohT16s.append(ohT16)
                    for t in range(tw):
                        nc.tensor.matmul(pads[:, t * heads:(t + 1) * heads],
                                         lhsT=ohT16s[t][:], rhs=ad_win,
                                         start=True, stop=True)
                    if ep < 4:
                        icol += kw // 16
                        ccol += tw
                        continue
                    psw = pp.tile([WIN, ncols], F32, tag="psw", bufs=2)
                    halves = [(g1a, 0, twa)]
                    if twb:
                        halves.append((g1b, twa, twb))
                    for gt, t0, twh in halves:
                        # per-half scores: p = exp(leaky(as+ad) - 3)
                        gt_3 = gt[:].rearrange("p (t e) -> p t e", e=row)
                        st = sp.tile([P, twh * heads], F32, tag="st")
                        st3 = st[:].rearrange("p (t h) -> p t h", h=heads)
                        nc.vector.tensor_tensor(
                            st3, gt_3[:, :, ncols:ncols + heads],
                            pads[:, t0 * heads:(t0 + twh) * heads].rearrange(
                                "p (t h) -> p t h", h=heads), op=ALU.add)
                        lt = sp.tile([P, twh * heads], F32, tag="lt")
                        nc.vector.tensor_scalar(lt[:], st[:], NEG, None, op0=ALU.mult)
                        nc.vector.tensor_tensor(lt[:], lt[:], st[:], op=ALU.max)
                        p16 = sp.tile([P, twh * heads], F16, tag="p16")
                        nc.scalar.activation(p16[:], lt[:], ACTF.Exp,
                                             bias=ebias[:, 0:1])
                        for tt in range(twh):
                            t = t0 + tt
                            rhs_t = rp.tile([P, ncols], F16, tag="rhs")
                            nc.vector.tensor_tensor(
                                rhs_t[:].rearrange("p (h c) -> p h c", c=65),
                                gt[:, tt * row:tt * row + ncols].rearrange(
                                    "p (h c) -> p h c", c=65),
                                p16[:, tt * heads:(tt + 1) * heads].broadcast_to(
                                    (P, heads, 65)),
                                op=ALU.mult)
                            nc.tensor.matmul(psw[:], lhsT=ohs[t][:], rhs=rhs_t[:],
                                             start=(t == 0), stop=(t == tw - 1))
                    if ep < 5:
                        icol += kw // 16
                        ccol += tw
                        continue
                    # window epilogue: normalize, bias(+relu), transpose for next
                    psw_v = psw[:].rearrange("p (h c) -> p h c", c=65)
                    den = sp.tile([WIN, heads], F32, tag="den")
                    nc.vector.tensor_scalar(
                        den[:].rearrange("p (h c) -> p h c", c=1),
                        psw_v[:, :, 64:65], 1e-16, None, op0=ALU.add)
                    rec = sp.tile([WIN, heads], F32, tag="rec")
                    nc.vector.reciprocal(rec[:], den[:])
                    if l < 2:
                        orow = op_.tile([P, fout], F32, tag="orow")
                        nc.vector.tensor_tensor(
                            orow[:WIN, :].rearrange("p (h c) -> p h c", c=64),
                            psw_v[:, :, 0:64],
                            rec[:, :].broadcast_to((WIN, heads, 64)), op=ALU.mult)
                        for j in range(fout // P):
                            pt = pp.tile([P, WIN], F32, tag="pt", bufs=1)
                            nc.tensor.transpose(pt[:], orow[:WIN, j * P:(j + 1) * P],
                                                ident[:WIN, :WIN])
                            # bias + relu ride the PSUM->SBUF copy
                            nc.vector.tensor_scalar(
                                actT_next[j][:, w * WIN:(w + 1) * WIN],
                                pt[:, :WIN], btTt[l][:, j:j + 1], 0.0,
                                op0=ALU.add, op1=ALU.max)
                    else:
                        orow = op_.tile([P, OUT_CH], F32, tag="orow")
                        nc.vector.tensor_scalar(orow[:WIN, :], psw[:, 0:OUT_CH],
                                                rec[:, 0:1], None, op0=ALU.mult)
                        nc.vector.tensor_tensor(orow[:WIN, :], orow[:WIN, :],
                                                bt3[:WIN, :], op=ALU.add)
                        nc.sync.dma_start(d_out[w * WIN:(w + 1) * WIN, :],
                                          orow[:WIN, :])
                    icol += kw // 16
                    ccol += tw

            def body():
                stages = int(os.environ.get("GAT_STAGES", "9"))  # 9 = full network
                dense_phase(0, [xT])
                if stages >= 2:
                    allgather(0)
                if stages >= 3:
                    edge_phase(0, actT[1])
                if stages >= 4:
                    dense_phase(1, actT[1])
                    allgather(1)
                if stages >= 5:
                    edge_phase(1, actT[2])
                if stages >= 6:
                    dense_phase(2, actT[2])
                    allgather(2)
                if stages >= 7:
                    edge_phase(2, None)
                if stages < 7:
                    z = op_.tile([WIN, OUT_CH], F32, tag="z", name="z")
                    nc.vector.memset(z[:], 0.0)
                    for w in range(NW):
                        nc.sync.dma_start(d_out[w * WIN:(w + 1) * WIN, :], z[:])

            if timing_reps:
                tk = cp.tile([1, 32], F32)
                nc.sync.dma_start(tk[:], d_tok[:, :])
                if timing_reps == 1:
                    body()
                else:
                    with tc.For_i(0, timing_reps, 1):
                        body()
                nc.sync.dma_start(d_toko[:, :], tk[:])
            else:
                body()

    nc.compile()
    return nc


def _host_inputs(x, edge_index, W1, a1s, a1d, b1, W2, a2s, a2d, b2, W3, a3s, a3d, b3):
    kws, isrc_all, idst_all, dl_all = _host_prep(edge_index)
    x = np.asarray(x, dtype=np.float32)
    Ws = [np.asarray(W1, np.float32), np.asarray(W2, np.float32),
          np.asarray(W3, np.float32)]
    As = [np.asarray(a1s, np.float32), np.asarray(a2s, np.float32),
          np.asarray(a3s, np.float32)]
    Ad = [np.asarray(a1d, np.float32), np.asarray(a2d, np.float32),
          np.asarray(a3d, np.float32)]
    bs = [np.asarray(b1, np.float32), np.asarray(b2, np.float32),
          np.asarray(b3, np.float32)]
    shared = {}
    for l in range(3):
        fout, heads = LAYERS[l]['fout'], LAYERS[l]['heads']
        dh = fout // heads
        # Wa[:, h] = W[:, h*dh:(h+1)*dh] @ a[h]  (as/ad folded into the dense matmul)
        W3d = Ws[l].reshape(-1, heads, dh)
        Was = np.einsum('ihd,hd->ih', W3d, As[l])
        Wad = np.einsum('ihd,hd->ih', W3d, Ad[l])
        shared[f"Wp{l+1}"] = np.concatenate([Ws[l], Was, Wad], axis=1).astype(np.float16)
    for l in range(2):
        shared[f"btT{l+1}"] = np.ascontiguousarray(
            bs[l].reshape(-1, P).T).astype(np.float32)
    shared["bt3"] = np.tile(bs[2].reshape(1, OUT_CH), (P, 1)).astype(np.float32)
    shared["iota32"] = np.tile(np.arange(P, dtype=np.float32).reshape(1, P), (P, 1))
    shared["ident"] = np.eye(P, dtype=np.float32)
    in_maps = []
    for c in range(C):
        m = dict(shared)
        m["xT_own"] = np.ascontiguousarray(
            x[c * SHARD:(c + 1) * SHARD].T).astype(np.float16)
        m["idx_src"] = isrc_all[c]
        m["idx_dst"] = idst_all[c]
        m["dl32"] = dl_all[c]
        in_maps.append(m)
    return kws, in_maps


_CACHE = {}


def kernel(**inputs) -> np.ndarray:
    kws, in_maps = _host_inputs(**inputs)
    if kws not in _CACHE:
        _CACHE[kws] = build(kws)
    nc = _CACHE[kws]
    last = None
    for _attempt in range(2):
        try:
            res = bass_utils.run_bass_kernel_spmd(
                nc, in_maps, core_ids=list(range(C)), trace=False)
            return np.concatenate(
                [res.results[c]["out"] for c in range(C)], axis=0)
        except Exception as e:  # rare transient device-mesh hiccups: retry once
            last = e
    raise last
